# revision 9
# baseline (speedup 1.0000x reference)
"""Trainium2 Bass kernel for DeLanJacobianNet inverse dynamics.

Strategy: pure data-parallel over 8 NeuronCores (4096 samples each).
Per core, activations live transposed (features on SBUF partitions, batch on
the free dim, 512-sample tiles). Every linear layer is a weight-stationary
PE matmul; the per-sample quadratic contractions (H / dH / grad terms)
become full-tile elementwise products plus matmuls against small constant
matrices (Wp, ones-blockdiagonal reduce/replicate patterns, index-mod-7
reduction patterns) that are folded host-side.

Layout: logical 3x7 "flat" blocks and 30-unit hidden blocks are padded to
32 partitions; tiles hold 4 blocks = 128 partitions:
  _lo tiles: [Jp, Jo0, Jo1, Jo2]   _hi tiles: [Jo3, Jo4, Jo5, Jo6]
sin/cos arguments are range-reduced with a single add_range_wrap (data max
|z| = 7.95 < 3pi; cos arg wraps the already-wrapped u by +pi/2).
"""
import numpy as np

DOF, HID_J, HID_G, EPS = 7, 30, 20, 1e-6
B_TOTAL = 32768
N_CORES = 8
BC = B_TOTAL // N_CORES          # samples per core
BT = 512                         # samples per tile (matmul N / PSUM bank)
_f32 = np.float32
_rows3, _cols3 = np.tril_indices(3)

CONST_SHAPES = {
    'cZS': (128, 128),          # stacked: rows 0-7 Z_lo, 32-39 Z_hi, 64-70 dZ_lo, 96-102 dZ_hi
    'cZ_g': (8, 20),
    'cFJ_lo': (128, 128), 'cFJ_hi': (128, 128),
    'cCMB_lo': (128, 128), 'cCMB_hi': (128, 128),
    'cRED_lo': (128, 128), 'cRED_hi': (128, 128),
    'cREDJ': (128, 7),
    'cBW_lo': (128, 128), 'cBW_hi': (128, 128),
    'cW1T_lo': (128, 7), 'cW1T_hi': (128, 7),
    'cG2W': (20, 7),
    'cFJb_lo': (128, 1), 'cFJb_hi': (128, 1),
    'cCMBb_lo': (128, 1), 'cCMBb_hi': (128, 1),
    'cgb2': (7, 1),
}


def _bd(blocks):
    h = sum(b.shape[0] for b in blocks)
    w = sum(b.shape[1] for b in blocks)
    out = np.zeros((h, w), _f32)
    r = c = 0
    for b in blocks:
        out[r:r + b.shape[0], c:c + b.shape[1]] = b
        r += b.shape[0]
        c += b.shape[1]
    return out


def _pad2d(m, rows, cols):
    out = np.zeros((rows, cols), _f32)
    out[:m.shape[0], :m.shape[1]] = m
    return out


def precompute_consts(inp):
    """Fold all small-weight preprocessing into the constant matrices the
    kernel consumes. Returns dict matching CONST_SHAPES."""
    J7 = np.ones((7, 7), _f32)
    m_param = np.asarray(inp['m_param'], _f32)
    inertia = np.asarray(inp['inertia'], _f32)
    mc = np.maximum(m_param, EPS)
    rev = np.cumsum(mc[::-1])[::-1]
    idx = np.arange(DOF)
    Wp = rev[np.maximum(idx[:, None], idx[None, :])].astype(_f32)

    Il = np.zeros((DOF, 3, 3), _f32)
    Il[:, _rows3, _cols3] = inertia
    I_mat = np.einsum('nab,ncb->nac', Il, Il).astype(_f32)

    JpW1 = np.asarray(inp['Jp_W1'], _f32); Jpb1 = np.asarray(inp['Jp_b1'], _f32)
    JpW2 = np.asarray(inp['Jp_W2'], _f32); Jpb2 = np.asarray(inp['Jp_b2'], _f32)
    JoW1 = np.asarray(inp['Jo_W1'], _f32); Job1 = np.asarray(inp['Jo_b1'], _f32)
    JoW2 = np.asarray(inp['Jo_W2'], _f32); Job2 = np.asarray(inp['Jo_b2'], _f32)
    gW1 = np.asarray(inp['g_W1'], _f32); gb1 = np.asarray(inp['g_b1'], _f32)
    gW2 = np.asarray(inp['g_W2'], _f32); gb2 = np.asarray(inp['g_b2'], _f32)

    KW2 = np.einsum('nlm,nhmj->nhlj', I_mat,
                    JoW2.reshape(7, HID_J, 3, 7)).reshape(7, HID_J, 21)
    Kb2 = np.einsum('nlm,nmj->nlj', I_mat,
                    Job2.reshape(7, 3, 7)).reshape(7, 21)

    nets_lo = [(JpW1, Jpb1, JpW2, Jpb2)] + \
        [(JoW1[n], Job1[n], JoW2[n], Job2[n]) for n in range(3)]
    nets_hi = [(JoW1[n], Job1[n], JoW2[n], Job2[n]) for n in range(3, 7)]

    def z_lhsT(nets, with_bias):
        out = np.zeros((8 if with_bias else 7, 32 * len(nets)), _f32)
        for b, (W1, b1, _, _) in enumerate(nets):
            out[0:7, 32 * b:32 * b + 30] = W1
            if with_bias:
                out[7, 32 * b:32 * b + 30] = b1
        return out

    C = {}
    czs = np.zeros((128, 128), _f32)
    czs[0:8, :] = z_lhsT(nets_lo, True)
    czs[32:40, :] = z_lhsT(nets_hi, True)
    czs[64:71, :] = z_lhsT(nets_lo, False)
    czs[96:103, :] = z_lhsT(nets_hi, False)
    C['cZS'] = czs
    C['cZ_g'] = np.concatenate([gW1, gb1[None, :]], 0)

    C['cFJ_lo'] = _bd([_pad2d(W2, 32, 32) for _, _, W2, _ in nets_lo])
    C['cFJ_hi'] = _bd([_pad2d(W2, 32, 32) for _, _, W2, _ in nets_hi])
    C['cFJb_lo'] = np.concatenate(
        [np.pad(b2, (0, 11)) for _, _, _, b2 in nets_lo])[:, None]
    C['cFJb_hi'] = np.concatenate(
        [np.pad(b2, (0, 11)) for _, _, _, b2 in nets_hi])[:, None]

    C['cCMB_lo'] = _bd([_pad2d(JpW2, 32, 32)] +
                       [_pad2d(KW2[n], 32, 32) for n in range(3)])
    C['cCMB_hi'] = _bd([_pad2d(KW2[n], 32, 32) for n in range(3, 7)])
    C['cCMBb_lo'] = np.concatenate(
        [np.pad(Jpb2, (0, 11))] +
        [np.pad(Kb2[n], (0, 11)) for n in range(3)])[:, None]
    C['cCMBb_hi'] = np.concatenate(
        [np.pad(Kb2[n], (0, 11)) for n in range(3, 7)])[:, None]

    WpI3p = _pad2d(np.kron(np.eye(3, dtype=_f32), Wp), 32, 32)
    RB = _pad2d(_bd([J7] * 3), 32, 32)
    C['cRED_lo'] = _bd([WpI3p, RB, RB, RB])
    C['cRED_hi'] = _bd([RB, RB, RB, RB])

    redj = np.zeros((128, 7), _f32)
    for r in range(128):
        if r % 32 < 21:
            redj[r, (r % 32) % 7] = 1.0
    C['cREDJ'] = redj

    C['cBW_lo'] = _bd([_pad2d(2 * W2.T, 32, 32) for _, _, W2, _ in nets_lo])
    C['cBW_hi'] = _bd([_pad2d(2 * W2.T, 32, 32) for _, _, W2, _ in nets_hi])

    w1t_lo = np.zeros((128, 7), _f32)
    for b, (W1, _, _, _) in enumerate(nets_lo):
        w1t_lo[32 * b:32 * b + 30, :] = W1.T
    C['cW1T_lo'] = w1t_lo
    w1t_hi = np.zeros((128, 7), _f32)
    for b, (W1, _, _, _) in enumerate(nets_hi):
        w1t_hi[32 * b:32 * b + 30, :] = W1.T
    C['cW1T_hi'] = w1t_hi

    C['cG2W'] = gW2
    C['cgb2'] = gb2[:, None]
    return {k: np.ascontiguousarray(v, dtype=_f32) for k, v in C.items()}


def build_host_inputs(x):
    """x [B,21] -> packed DMA-source arrays, each [32, B]:
    xqv1: rows 0-7 [q^T;1] (replicated to both q row-groups on chip)
    xqv2: rows 0-6 qDot^T
    xrep_a / xrep_v: row s = col s%7 of qDDot^T / qDot^T (per-block pattern)
    """
    x = np.asarray(x, _f32)
    B = x.shape[0]
    qT, vT, aT = x[:, 0:7].T, x[:, 7:14].T, x[:, 14:21].T
    xqv1 = np.zeros((32, B), _f32)
    xqv1[0:7] = qT
    xqv1[7] = 1.0
    xqv2 = np.zeros((32, B), _f32)
    xqv2[0:7] = vT
    idx = np.arange(32) % 7
    xrep_a = np.ascontiguousarray(aT[idx])
    xrep_v = np.ascontiguousarray(vT[idx])
    return {'xqv1': xqv1, 'xqv2': xqv2, 'xrep_a': xrep_a, 'xrep_v': xrep_v}


def build_program(bc=BC, bt=BT, debug=False):
    """Build + compile the per-core Bass program. Returns the Bacc object."""
    import concourse.bacc as bacc
    import concourse.tile as tile
    from concourse import mybir

    AF = mybir.ActivationFunctionType
    f32 = mybir.dt.float32
    PI = float(np.pi)

    nc = bacc.Bacc("TRN2", target_bir_lowering=False, debug=debug)
    xins = {n: nc.dram_tensor(n, [32, bc], f32, kind='ExternalInput').ap()
            for n in ('xqv1', 'xqv2', 'xrep_a', 'xrep_v')}
    outd = nc.dram_tensor('out', [7, bc], f32, kind='ExternalOutput').ap()
    dr = {n: nc.dram_tensor(n, list(s), f32, kind='ExternalInput').ap()
          for n, s in CONST_SHAPES.items()}

    import concourse.bass as bass

    with tile.TileContext(nc) as tc:
        with tc.tile_pool(name='consts', bufs=1) as cp, \
             tc.tile_pool(name='work', bufs=2) as wp, \
             tc.tile_pool(name='ps', bufs=7, space='PSUM') as ps, \
             tc.tile_pool(name='pso', bufs=1, space='PSUM') as pso:

            ct = {}
            for n, s in CONST_SHAPES.items():
                t = cp.tile(list(s), f32, tag=n, name=n)
                nc.sync.dma_start(out=t, in_=dr[n])
                ct[n] = t

            def mmt(rows=128):
                return ps.tile([rows, bt], f32, tag='mm', name='mm')

            def wt(rows, tag):
                return wp.tile([rows, bt], f32, tag=tag, name=tag)

            for it in range(bc // bt):
                c0 = it * bt
                arep = wt(128, 'arep')
                vrep = wt(128, 'vrep')
                qv = wt(128, 'qv')
                for dst, srcn, nblk in [(arep, 'xrep_a', 4), (vrep, 'xrep_v', 4)]:
                    src = xins[srcn]
                    in_ = bass.AP(tensor=src.tensor, offset=src.offset + c0,
                                  ap=[[0, nblk], [bc, 32], [1, bt]])
                    nc.sync.dma_start(out=dst, in_=in_)
                in1 = bass.AP(tensor=xins['xqv1'].tensor,
                              offset=xins['xqv1'].offset + c0,
                              ap=[[0, 2], [bc, 32], [1, bt]])
                nc.sync.dma_start(out=qv[0:64], in_=in1)
                in2 = bass.AP(tensor=xins['xqv2'].tensor,
                              offset=xins['xqv2'].offset + c0,
                              ap=[[0, 2], [bc, 32], [1, bt]])
                nc.sync.dma_start(out=qv[64:128], in_=in2)

                # ---- first layer: 4 packed matmuls (row groups) + g ----
                z_lo = mmt(); z_hi = mmt(); dz_lo = mmt(); dz_hi = mmt()
                nc.tensor.matmul(z_lo, ct['cZS'][0:8], qv[0:8],
                                 start=True, stop=True, tile_position=(0, 0))
                nc.tensor.matmul(z_hi, ct['cZS'][32:40], qv[32:40],
                                 start=True, stop=True, tile_position=(32, 0))
                nc.tensor.matmul(dz_lo, ct['cZS'][64:71], qv[64:71],
                                 start=True, stop=True, tile_position=(64, 0))
                nc.tensor.matmul(dz_hi, ct['cZS'][96:103], qv[96:103],
                                 start=True, stop=True, tile_position=(96, 0))
                z_g = mmt(20)
                nc.tensor.matmul(z_g, ct['cZ_g'], qv[0:8], start=True, stop=True)

                # ---- range reduction + sin/cos ----
                u_lo = wt(128, 'u_lo'); nc.vector.add_range_wrap(u_lo, z_lo, 0.0, PI, 2 * PI)
                u_hi = wt(128, 'u_hi'); nc.vector.add_range_wrap(u_hi, z_hi, 0.0, PI, 2 * PI)
                u_g = wt(20, 'u_g'); nc.vector.add_range_wrap(u_g, z_g, 0.0, PI, 2 * PI)
                s_lo = wt(128, 's_lo'); nc.scalar.activation(s_lo, u_lo, AF.Sin)
                s_hi = wt(128, 's_hi'); nc.scalar.activation(s_hi, u_hi, AF.Sin)
                s_g = wt(20, 's_g'); nc.scalar.activation(s_g, u_g, AF.Sin)
                u2_lo = wt(128, 'u2_lo'); nc.vector.add_range_wrap(u2_lo, u_lo, PI / 2, PI, 2 * PI)
                u2_hi = wt(128, 'u2_hi'); nc.vector.add_range_wrap(u2_hi, u_hi, PI / 2, PI, 2 * PI)
                cs_lo = wt(128, 'cs_lo'); nc.scalar.activation(cs_lo, u2_lo, AF.Sin)
                cs_hi = wt(128, 'cs_hi'); nc.scalar.activation(cs_hi, u2_hi, AF.Sin)

                ds_lo = wt(128, 'ds_lo'); nc.vector.tensor_mul(ds_lo, cs_lo, dz_lo)
                ds_hi = wt(128, 'ds_hi'); nc.vector.tensor_mul(ds_hi, cs_hi, dz_hi)

                # ---- output accumulator: gravity net first ----
                outp = pso.tile([7, bt], f32, tag='out', name='outp')
                nc.tensor.matmul(outp, ct['cG2W'], s_g, start=True, stop=False)

                # ---- second layer ----
                fj_lo_p = mmt(); nc.tensor.matmul(fj_lo_p, ct['cFJ_lo'], s_lo, start=True, stop=True)
                fj_lo = wt(128, 'fj_lo'); nc.scalar.activation(fj_lo, fj_lo_p, AF.Identity, bias=ct['cFJb_lo'])
                fj_hi_p = mmt(); nc.tensor.matmul(fj_hi_p, ct['cFJ_hi'], s_hi, start=True, stop=True)
                fj_hi = wt(128, 'fj_hi'); nc.scalar.activation(fj_hi, fj_hi_p, AF.Identity, bias=ct['cFJb_hi'])
                cmb_lo_p = mmt(); nc.tensor.matmul(cmb_lo_p, ct['cCMB_lo'], s_lo, start=True, stop=True)
                cmb_lo = wt(128, 'cmb_lo'); nc.scalar.activation(cmb_lo, cmb_lo_p, AF.Identity, bias=ct['cCMBb_lo'])
                cmb_hi_p = mmt(); nc.tensor.matmul(cmb_hi_p, ct['cCMB_hi'], s_hi, start=True, stop=True)
                cmb_hi = wt(128, 'cmb_hi'); nc.scalar.activation(cmb_hi, cmb_hi_p, AF.Identity, bias=ct['cCMBb_hi'])

                dfj_lo = mmt(); nc.tensor.matmul(dfj_lo, ct['cFJ_lo'], ds_lo, start=True, stop=True)
                dfj_hi = mmt(); nc.tensor.matmul(dfj_hi, ct['cFJ_hi'], ds_hi, start=True, stop=True)
                dcmb_lo_p = mmt(); nc.tensor.matmul(dcmb_lo_p, ct['cCMB_lo'], ds_lo, start=True, stop=True)
                dcmb_lo = wt(128, 'dcmb_lo'); nc.scalar.copy(dcmb_lo, dcmb_lo_p)
                dcmb_hi_p = mmt(); nc.tensor.matmul(dcmb_hi_p, ct['cCMB_hi'], ds_hi, start=True, stop=True)
                dcmb_hi = wt(128, 'dcmb_hi'); nc.scalar.copy(dcmb_hi, dcmb_hi_p)

                # ---- quadratic-form products (gpsimd: SBUF-only; DVE: PSUM) ----
                xa_lo = wt(128, 'xa_lo'); nc.gpsimd.tensor_mul(xa_lo, fj_lo, arep)
                xa_hi = wt(128, 'xa_hi'); nc.gpsimd.tensor_mul(xa_hi, fj_hi, arep)
                xb_lo = wt(128, 'xb_lo'); nc.vector.tensor_mul(xb_lo, vrep, dfj_lo)
                xb_hi = wt(128, 'xb_hi'); nc.vector.tensor_mul(xb_hi, vrep, dfj_hi)
                xs_lo = wt(128, 'xs_lo'); nc.vector.tensor_add(xs_lo, xa_lo, xb_lo)
                xs_hi = wt(128, 'xs_hi'); nc.vector.tensor_add(xs_hi, xa_hi, xb_hi)
                p_lo = wt(128, 'p_lo'); nc.vector.tensor_mul(p_lo, fj_lo, vrep)
                p_hi = wt(128, 'p_hi'); nc.vector.tensor_mul(p_hi, fj_hi, vrep)
                kv_lo = wt(128, 'kv_lo'); nc.vector.tensor_mul(kv_lo, cmb_lo, vrep)
                kv_hi = wt(128, 'kv_hi'); nc.vector.tensor_mul(kv_hi, cmb_hi, vrep)

                # ---- constant-pattern folds on PE ----
                r2_lo = mmt(); nc.tensor.matmul(r2_lo, ct['cRED_lo'], xs_lo, start=True, stop=True)
                r2_hi = mmt(); nc.tensor.matmul(r2_hi, ct['cRED_hi'], xs_hi, start=True, stop=True)
                pw_lo = mmt(); nc.tensor.matmul(pw_lo, ct['cRED_lo'], p_lo, start=True, stop=True)
                pw_hi = mmt(); nc.tensor.matmul(pw_hi, ct['cRED_hi'], p_hi, start=True, stop=True)
                kpw_lo = mmt(); nc.tensor.matmul(kpw_lo, ct['cRED_lo'], kv_lo, start=True, stop=True)
                kpw_hi = mmt(); nc.tensor.matmul(kpw_hi, ct['cRED_hi'], kv_hi, start=True, stop=True)

                g1_lo = wt(128, 'g1_lo'); nc.vector.tensor_mul(g1_lo, cmb_lo, r2_lo)
                g1_hi = wt(128, 'g1_hi'); nc.vector.tensor_mul(g1_hi, cmb_hi, r2_hi)
                g2d_lo = wt(128, 'g2d_lo'); nc.vector.tensor_mul(g2d_lo, dcmb_lo, pw_lo)
                g2d_hi = wt(128, 'g2d_hi'); nc.vector.tensor_mul(g2d_hi, dcmb_hi, pw_hi)
                y_lo = wt(128, 'y_lo'); nc.vector.tensor_add(y_lo, g1_lo, g2d_lo)
                y_hi = wt(128, 'y_hi'); nc.vector.tensor_add(y_hi, g1_hi, g2d_hi)
                u_t_lo = wt(128, 'u_t_lo'); nc.vector.tensor_mul(u_t_lo, vrep, kpw_lo)
                u_t_hi = wt(128, 'u_t_hi'); nc.vector.tensor_mul(u_t_hi, vrep, kpw_hi)

                nc.tensor.matmul(outp, ct['cREDJ'], y_lo, start=False, stop=False)
                nc.tensor.matmul(outp, ct['cREDJ'], y_hi, start=False, stop=False)

                # ---- gradient backprop ----
                bh_lo = mmt(); nc.tensor.matmul(bh_lo, ct['cBW_lo'], u_t_lo, start=True, stop=True)
                bhc_lo = wt(128, 'bhc_lo'); nc.vector.tensor_mul(bhc_lo, cs_lo, bh_lo)
                bh_hi = mmt(); nc.tensor.matmul(bh_hi, ct['cBW_hi'], u_t_hi, start=True, stop=True)
                bhc_hi = wt(128, 'bhc_hi'); nc.vector.tensor_mul(bhc_hi, cs_hi, bh_hi)

                nc.tensor.matmul(outp, ct['cW1T_lo'], bhc_lo, start=False, stop=False)
                nc.tensor.matmul(outp, ct['cW1T_hi'], bhc_hi, start=False, stop=True)

                res = wt(7, 'res')
                nc.vector.tensor_scalar_add(res, outp, ct['cgb2'])
                nc.sync.dma_start(out=outd[:, c0:c0 + bt], in_=res)

    nc.compile()
    return nc


_PROGRAM_CACHE = {}


def kernel(**inputs):
    x = np.asarray(inputs['x'], _f32)
    assert x.shape == (B_TOTAL, 3 * DOF), x.shape
    C = precompute_consts(inputs)
    hostin = build_host_inputs(x)

    key = (BC, BT)
    if key not in _PROGRAM_CACHE:
        _PROGRAM_CACHE[key] = build_program(BC, BT)
    nc = _PROGRAM_CACHE[key]

    from concourse.bass_utils import run_bass_kernel_spmd
    in_maps = []
    for i in range(N_CORES):
        m = dict(C)
        for n, arr in hostin.items():
            m[n] = np.ascontiguousarray(arr[:, i * BC:(i + 1) * BC])
        in_maps.append(m)
    res = run_bass_kernel_spmd(nc, in_maps, core_ids=list(range(N_CORES))).results
    out = np.concatenate([res[i]['out'] for i in range(N_CORES)], axis=1)  # [7,B]
    return np.ascontiguousarray(out.T).astype(_f32)


# revision 10
# speedup vs baseline: 1.0315x; 1.0315x over previous
"""Trainium2 Bass kernel for DeLanJacobianNet inverse dynamics.

Strategy: pure data-parallel over 8 NeuronCores (4096 samples each).
Per core, activations live transposed (features on SBUF partitions, batch on
the free dim, 512-sample tiles). Every linear layer is a weight-stationary
PE matmul; the per-sample quadratic contractions (H / dH / grad terms)
become full-tile elementwise products plus matmuls against small constant
matrices (Wp, ones-blockdiagonal reduce/replicate patterns, index-mod-7
reduction patterns) that are folded host-side.

Layout: logical 3x7 "flat" blocks and 30-unit hidden blocks are padded to
32 partitions; tiles hold 4 blocks = 128 partitions:
  _lo tiles: [Jp, Jo0, Jo1, Jo2]   _hi tiles: [Jo3, Jo4, Jo5, Jo6]
sin/cos arguments are range-reduced with a single add_range_wrap (data max
|z| = 7.95 < 3pi; cos arg wraps the already-wrapped u by +pi/2).
"""
import numpy as np

DOF, HID_J, HID_G, EPS = 7, 30, 20, 1e-6
B_TOTAL = 32768
N_CORES = 8
BC = B_TOTAL // N_CORES          # samples per core
BT = 512                         # samples per tile (matmul N / PSUM bank)
_f32 = np.float32
_rows3, _cols3 = np.tril_indices(3)

CONST_SHAPES = {
    'cZS': (128, 128),          # stacked: rows 0-7 Z_lo, 32-39 Z_hi, 64-70 dZ_lo, 96-102 dZ_hi
    'cZ_g': (8, 20),
    'cFJ_lo': (128, 128), 'cFJ_hi': (128, 128),
    'cCMB_lo': (128, 128), 'cCMB_hi': (128, 128),
    'cRED_lo': (128, 128), 'cRED_hi': (128, 128),
    'cREDJ': (128, 7),
    'cBW_lo': (128, 128), 'cBW_hi': (128, 128),
    'cW1T_lo': (128, 7), 'cW1T_hi': (128, 7),
    'cG2W': (20, 7),
    'cFJb_lo': (128, 1), 'cFJb_hi': (128, 1),
    'cCMBb_lo': (128, 1), 'cCMBb_hi': (128, 1),
    'cgb2': (7, 1),
}


def _pack_layout():
    off = 0
    lay = {}
    for n, (r, c) in CONST_SHAPES.items():
        lay[n] = (off, r, c)
        off += c
    return lay, off


PACK_LAYOUT, PACK_COLS = _pack_layout()


def pack_consts(C):
    out = np.zeros((128, PACK_COLS), _f32)
    for n, (off, r, c) in PACK_LAYOUT.items():
        out[0:r, off:off + c] = C[n]
    return out


def _bd(blocks):
    h = sum(b.shape[0] for b in blocks)
    w = sum(b.shape[1] for b in blocks)
    out = np.zeros((h, w), _f32)
    r = c = 0
    for b in blocks:
        out[r:r + b.shape[0], c:c + b.shape[1]] = b
        r += b.shape[0]
        c += b.shape[1]
    return out


def _pad2d(m, rows, cols):
    out = np.zeros((rows, cols), _f32)
    out[:m.shape[0], :m.shape[1]] = m
    return out


def precompute_consts(inp):
    """Fold all small-weight preprocessing into the constant matrices the
    kernel consumes. Returns dict matching CONST_SHAPES."""
    J7 = np.ones((7, 7), _f32)
    m_param = np.asarray(inp['m_param'], _f32)
    inertia = np.asarray(inp['inertia'], _f32)
    mc = np.maximum(m_param, EPS)
    rev = np.cumsum(mc[::-1])[::-1]
    idx = np.arange(DOF)
    Wp = rev[np.maximum(idx[:, None], idx[None, :])].astype(_f32)

    Il = np.zeros((DOF, 3, 3), _f32)
    Il[:, _rows3, _cols3] = inertia
    I_mat = np.einsum('nab,ncb->nac', Il, Il).astype(_f32)

    JpW1 = np.asarray(inp['Jp_W1'], _f32); Jpb1 = np.asarray(inp['Jp_b1'], _f32)
    JpW2 = np.asarray(inp['Jp_W2'], _f32); Jpb2 = np.asarray(inp['Jp_b2'], _f32)
    JoW1 = np.asarray(inp['Jo_W1'], _f32); Job1 = np.asarray(inp['Jo_b1'], _f32)
    JoW2 = np.asarray(inp['Jo_W2'], _f32); Job2 = np.asarray(inp['Jo_b2'], _f32)
    gW1 = np.asarray(inp['g_W1'], _f32); gb1 = np.asarray(inp['g_b1'], _f32)
    gW2 = np.asarray(inp['g_W2'], _f32); gb2 = np.asarray(inp['g_b2'], _f32)

    KW2 = np.einsum('nlm,nhmj->nhlj', I_mat,
                    JoW2.reshape(7, HID_J, 3, 7)).reshape(7, HID_J, 21)
    Kb2 = np.einsum('nlm,nmj->nlj', I_mat,
                    Job2.reshape(7, 3, 7)).reshape(7, 21)

    nets_lo = [(JpW1, Jpb1, JpW2, Jpb2)] + \
        [(JoW1[n], Job1[n], JoW2[n], Job2[n]) for n in range(3)]
    nets_hi = [(JoW1[n], Job1[n], JoW2[n], Job2[n]) for n in range(3, 7)]

    def z_lhsT(nets, with_bias):
        out = np.zeros((8 if with_bias else 7, 32 * len(nets)), _f32)
        for b, (W1, b1, _, _) in enumerate(nets):
            out[0:7, 32 * b:32 * b + 30] = W1
            if with_bias:
                out[7, 32 * b:32 * b + 30] = b1
        return out

    C = {}
    czs = np.zeros((128, 128), _f32)
    czs[0:8, :] = z_lhsT(nets_lo, True)
    czs[32:40, :] = z_lhsT(nets_hi, True)
    czs[64:71, :] = z_lhsT(nets_lo, False)
    czs[96:103, :] = z_lhsT(nets_hi, False)
    C['cZS'] = czs
    C['cZ_g'] = np.concatenate([gW1, gb1[None, :]], 0)

    C['cFJ_lo'] = _bd([_pad2d(W2, 32, 32) for _, _, W2, _ in nets_lo])
    C['cFJ_hi'] = _bd([_pad2d(W2, 32, 32) for _, _, W2, _ in nets_hi])
    C['cFJb_lo'] = np.concatenate(
        [np.pad(b2, (0, 11)) for _, _, _, b2 in nets_lo])[:, None]
    C['cFJb_hi'] = np.concatenate(
        [np.pad(b2, (0, 11)) for _, _, _, b2 in nets_hi])[:, None]

    C['cCMB_lo'] = _bd([_pad2d(JpW2, 32, 32)] +
                       [_pad2d(KW2[n], 32, 32) for n in range(3)])
    C['cCMB_hi'] = _bd([_pad2d(KW2[n], 32, 32) for n in range(3, 7)])
    C['cCMBb_lo'] = np.concatenate(
        [np.pad(Jpb2, (0, 11))] +
        [np.pad(Kb2[n], (0, 11)) for n in range(3)])[:, None]
    C['cCMBb_hi'] = np.concatenate(
        [np.pad(Kb2[n], (0, 11)) for n in range(3, 7)])[:, None]

    WpI3p = _pad2d(np.kron(np.eye(3, dtype=_f32), Wp), 32, 32)
    RB = _pad2d(_bd([J7] * 3), 32, 32)
    C['cRED_lo'] = _bd([WpI3p, RB, RB, RB])
    C['cRED_hi'] = _bd([RB, RB, RB, RB])

    redj = np.zeros((128, 7), _f32)
    for r in range(128):
        if r % 32 < 21:
            redj[r, (r % 32) % 7] = 1.0
    C['cREDJ'] = redj

    C['cBW_lo'] = _bd([_pad2d(2 * W2.T, 32, 32) for _, _, W2, _ in nets_lo])
    C['cBW_hi'] = _bd([_pad2d(2 * W2.T, 32, 32) for _, _, W2, _ in nets_hi])

    w1t_lo = np.zeros((128, 7), _f32)
    for b, (W1, _, _, _) in enumerate(nets_lo):
        w1t_lo[32 * b:32 * b + 30, :] = W1.T
    C['cW1T_lo'] = w1t_lo
    w1t_hi = np.zeros((128, 7), _f32)
    for b, (W1, _, _, _) in enumerate(nets_hi):
        w1t_hi[32 * b:32 * b + 30, :] = W1.T
    C['cW1T_hi'] = w1t_hi

    C['cG2W'] = gW2
    C['cgb2'] = gb2[:, None]
    return {k: np.ascontiguousarray(v, dtype=_f32) for k, v in C.items()}


def build_host_inputs(x):
    """x [B,21] -> packed DMA-source arrays, each [32, B]:
    xqv1: rows 0-7 [q^T;1] (replicated to both q row-groups on chip)
    xqv2: rows 0-6 qDot^T
    xrep_a / xrep_v: row s = col s%7 of qDDot^T / qDot^T (per-block pattern)
    """
    x = np.asarray(x, _f32)
    B = x.shape[0]
    qT, vT, aT = x[:, 0:7].T, x[:, 7:14].T, x[:, 14:21].T
    xqv1 = np.zeros((32, B), _f32)
    xqv1[0:7] = qT
    xqv1[7] = 1.0
    xqv2 = np.zeros((32, B), _f32)
    xqv2[0:7] = vT
    idx = np.arange(32) % 7
    xrep_a = np.ascontiguousarray(aT[idx])
    xrep_v = np.ascontiguousarray(vT[idx])
    return {'xqv1': xqv1, 'xqv2': xqv2, 'xrep_a': xrep_a, 'xrep_v': xrep_v}


def build_program(bc=BC, bt=BT, debug=False):
    """Build + compile the per-core Bass program. Returns the Bacc object."""
    import concourse.bacc as bacc
    import concourse.tile as tile
    from concourse import mybir

    AF = mybir.ActivationFunctionType
    f32 = mybir.dt.float32
    PI = float(np.pi)

    nc = bacc.Bacc("TRN2", target_bir_lowering=False, debug=debug)
    xins = {n: nc.dram_tensor(n, [32, bc], f32, kind='ExternalInput').ap()
            for n in ('xqv1', 'xqv2', 'xrep_a', 'xrep_v')}
    outd = nc.dram_tensor('out', [7, bc], f32, kind='ExternalOutput').ap()
    cpackd = nc.dram_tensor('cpack', [128, PACK_COLS], f32,
                            kind='ExternalInput').ap()

    import concourse.bass as bass

    with tile.TileContext(nc) as tc:
        with tc.tile_pool(name='consts', bufs=1) as cp, \
             tc.tile_pool(name='work', bufs=2) as wp, \
             tc.tile_pool(name='ps', bufs=7, space='PSUM') as ps, \
             tc.tile_pool(name='pso', bufs=1, space='PSUM') as pso:

            cpk = cp.tile([128, PACK_COLS], f32, tag='cpack', name='cpack')
            nc.sync.dma_start(out=cpk, in_=cpackd)
            ct = {n: cpk[0:r, off:off + c]
                  for n, (off, r, c) in PACK_LAYOUT.items()}

            def mmt(rows=128):
                return ps.tile([rows, bt], f32, tag='mm', name='mm')

            def wt(rows, tag):
                return wp.tile([rows, bt], f32, tag=tag, name=tag)

            for it in range(bc // bt):
                c0 = it * bt
                arep = wt(128, 'arep')
                vrep = wt(128, 'vrep')
                qv = wt(128, 'qv')
                for dst, srcn, nblk in [(arep, 'xrep_a', 4), (vrep, 'xrep_v', 4)]:
                    src = xins[srcn]
                    in_ = bass.AP(tensor=src.tensor, offset=src.offset + c0,
                                  ap=[[0, nblk], [bc, 32], [1, bt]])
                    nc.sync.dma_start(out=dst, in_=in_)
                in1 = bass.AP(tensor=xins['xqv1'].tensor,
                              offset=xins['xqv1'].offset + c0,
                              ap=[[0, 2], [bc, 32], [1, bt]])
                nc.sync.dma_start(out=qv[0:64], in_=in1)
                in2 = bass.AP(tensor=xins['xqv2'].tensor,
                              offset=xins['xqv2'].offset + c0,
                              ap=[[0, 2], [bc, 32], [1, bt]])
                nc.sync.dma_start(out=qv[64:128], in_=in2)

                # ---- first layer: 4 packed matmuls (row groups) + g ----
                z_lo = mmt(); z_hi = mmt(); dz_lo = mmt(); dz_hi = mmt()
                nc.tensor.matmul(z_lo, ct['cZS'][0:8], qv[0:8],
                                 start=True, stop=True, tile_position=(0, 0))
                nc.tensor.matmul(z_hi, ct['cZS'][32:40], qv[32:40],
                                 start=True, stop=True, tile_position=(32, 0))
                nc.tensor.matmul(dz_lo, ct['cZS'][64:71], qv[64:71],
                                 start=True, stop=True, tile_position=(64, 0))
                nc.tensor.matmul(dz_hi, ct['cZS'][96:103], qv[96:103],
                                 start=True, stop=True, tile_position=(96, 0))
                z_g = mmt(20)
                nc.tensor.matmul(z_g, ct['cZ_g'], qv[0:8], start=True, stop=True)

                # ---- range reduction + sin/cos ----
                u_lo = wt(128, 'u_lo'); nc.vector.add_range_wrap(u_lo, z_lo, 0.0, PI, 2 * PI)
                u_hi = wt(128, 'u_hi'); nc.vector.add_range_wrap(u_hi, z_hi, 0.0, PI, 2 * PI)
                u_g = wt(20, 'u_g'); nc.vector.add_range_wrap(u_g, z_g, 0.0, PI, 2 * PI)
                s_lo = wt(128, 's_lo'); nc.scalar.activation(s_lo, u_lo, AF.Sin)
                s_hi = wt(128, 's_hi'); nc.scalar.activation(s_hi, u_hi, AF.Sin)
                s_g = wt(20, 's_g'); nc.scalar.activation(s_g, u_g, AF.Sin)
                u2_lo = wt(128, 'u2_lo'); nc.vector.add_range_wrap(u2_lo, u_lo, PI / 2, PI, 2 * PI)
                u2_hi = wt(128, 'u2_hi'); nc.vector.add_range_wrap(u2_hi, u_hi, PI / 2, PI, 2 * PI)
                cs_lo = wt(128, 'cs_lo'); nc.scalar.activation(cs_lo, u2_lo, AF.Sin)
                cs_hi = wt(128, 'cs_hi'); nc.scalar.activation(cs_hi, u2_hi, AF.Sin)

                ds_lo = wt(128, 'ds_lo'); nc.vector.tensor_mul(ds_lo, cs_lo, dz_lo)
                ds_hi = wt(128, 'ds_hi'); nc.vector.tensor_mul(ds_hi, cs_hi, dz_hi)

                # ---- output accumulator: gravity net first ----
                outp = pso.tile([7, bt], f32, tag='out', name='outp')
                nc.tensor.matmul(outp, ct['cG2W'], s_g, start=True, stop=False)

                # ---- second layer ----
                fj_lo_p = mmt(); nc.tensor.matmul(fj_lo_p, ct['cFJ_lo'], s_lo, start=True, stop=True)
                fj_lo = wt(128, 'fj_lo'); nc.scalar.activation(fj_lo, fj_lo_p, AF.Identity, bias=ct['cFJb_lo'])
                fj_hi_p = mmt(); nc.tensor.matmul(fj_hi_p, ct['cFJ_hi'], s_hi, start=True, stop=True)
                fj_hi = wt(128, 'fj_hi'); nc.scalar.activation(fj_hi, fj_hi_p, AF.Identity, bias=ct['cFJb_hi'])
                cmb_lo_p = mmt(); nc.tensor.matmul(cmb_lo_p, ct['cCMB_lo'], s_lo, start=True, stop=True)
                cmb_lo = wt(128, 'cmb_lo'); nc.scalar.activation(cmb_lo, cmb_lo_p, AF.Identity, bias=ct['cCMBb_lo'])
                cmb_hi_p = mmt(); nc.tensor.matmul(cmb_hi_p, ct['cCMB_hi'], s_hi, start=True, stop=True)
                cmb_hi = wt(128, 'cmb_hi'); nc.scalar.activation(cmb_hi, cmb_hi_p, AF.Identity, bias=ct['cCMBb_hi'])

                dfj_lo = mmt(); nc.tensor.matmul(dfj_lo, ct['cFJ_lo'], ds_lo, start=True, stop=True)
                dfj_hi = mmt(); nc.tensor.matmul(dfj_hi, ct['cFJ_hi'], ds_hi, start=True, stop=True)
                dcmb_lo_p = mmt(); nc.tensor.matmul(dcmb_lo_p, ct['cCMB_lo'], ds_lo, start=True, stop=True)
                dcmb_lo = wt(128, 'dcmb_lo'); nc.scalar.copy(dcmb_lo, dcmb_lo_p)
                dcmb_hi_p = mmt(); nc.tensor.matmul(dcmb_hi_p, ct['cCMB_hi'], ds_hi, start=True, stop=True)
                dcmb_hi = wt(128, 'dcmb_hi'); nc.scalar.copy(dcmb_hi, dcmb_hi_p)

                # ---- quadratic-form products (gpsimd: SBUF-only; DVE: PSUM) ----
                xa_lo = wt(128, 'xa_lo'); nc.gpsimd.tensor_mul(xa_lo, fj_lo, arep)
                xa_hi = wt(128, 'xa_hi'); nc.gpsimd.tensor_mul(xa_hi, fj_hi, arep)
                xb_lo = wt(128, 'xb_lo'); nc.vector.tensor_mul(xb_lo, vrep, dfj_lo)
                xb_hi = wt(128, 'xb_hi'); nc.vector.tensor_mul(xb_hi, vrep, dfj_hi)
                xs_lo = wt(128, 'xs_lo'); nc.vector.tensor_add(xs_lo, xa_lo, xb_lo)
                xs_hi = wt(128, 'xs_hi'); nc.vector.tensor_add(xs_hi, xa_hi, xb_hi)
                p_lo = wt(128, 'p_lo'); nc.vector.tensor_mul(p_lo, fj_lo, vrep)
                p_hi = wt(128, 'p_hi'); nc.vector.tensor_mul(p_hi, fj_hi, vrep)
                kv_lo = wt(128, 'kv_lo'); nc.vector.tensor_mul(kv_lo, cmb_lo, vrep)
                kv_hi = wt(128, 'kv_hi'); nc.vector.tensor_mul(kv_hi, cmb_hi, vrep)

                # ---- constant-pattern folds on PE ----
                r2_lo = mmt(); nc.tensor.matmul(r2_lo, ct['cRED_lo'], xs_lo, start=True, stop=True)
                r2_hi = mmt(); nc.tensor.matmul(r2_hi, ct['cRED_hi'], xs_hi, start=True, stop=True)
                pw_lo = mmt(); nc.tensor.matmul(pw_lo, ct['cRED_lo'], p_lo, start=True, stop=True)
                pw_hi = mmt(); nc.tensor.matmul(pw_hi, ct['cRED_hi'], p_hi, start=True, stop=True)
                kpw_lo = mmt(); nc.tensor.matmul(kpw_lo, ct['cRED_lo'], kv_lo, start=True, stop=True)
                kpw_hi = mmt(); nc.tensor.matmul(kpw_hi, ct['cRED_hi'], kv_hi, start=True, stop=True)

                g1_lo = wt(128, 'g1_lo'); nc.vector.tensor_mul(g1_lo, cmb_lo, r2_lo)
                g1_hi = wt(128, 'g1_hi'); nc.vector.tensor_mul(g1_hi, cmb_hi, r2_hi)
                g2d_lo = wt(128, 'g2d_lo'); nc.vector.tensor_mul(g2d_lo, dcmb_lo, pw_lo)
                g2d_hi = wt(128, 'g2d_hi'); nc.vector.tensor_mul(g2d_hi, dcmb_hi, pw_hi)
                y_lo = wt(128, 'y_lo'); nc.vector.tensor_add(y_lo, g1_lo, g2d_lo)
                y_hi = wt(128, 'y_hi'); nc.vector.tensor_add(y_hi, g1_hi, g2d_hi)
                u_t_lo = wt(128, 'u_t_lo'); nc.vector.tensor_mul(u_t_lo, vrep, kpw_lo)
                u_t_hi = wt(128, 'u_t_hi'); nc.vector.tensor_mul(u_t_hi, vrep, kpw_hi)

                nc.tensor.matmul(outp, ct['cREDJ'], y_lo, start=False, stop=False)
                nc.tensor.matmul(outp, ct['cREDJ'], y_hi, start=False, stop=False)

                # ---- gradient backprop ----
                bh_lo = mmt(); nc.tensor.matmul(bh_lo, ct['cBW_lo'], u_t_lo, start=True, stop=True)
                bhc_lo = wt(128, 'bhc_lo'); nc.vector.tensor_mul(bhc_lo, cs_lo, bh_lo)
                bh_hi = mmt(); nc.tensor.matmul(bh_hi, ct['cBW_hi'], u_t_hi, start=True, stop=True)
                bhc_hi = wt(128, 'bhc_hi'); nc.vector.tensor_mul(bhc_hi, cs_hi, bh_hi)

                nc.tensor.matmul(outp, ct['cW1T_lo'], bhc_lo, start=False, stop=False)
                nc.tensor.matmul(outp, ct['cW1T_hi'], bhc_hi, start=False, stop=True)

                res = wt(7, 'res')
                nc.vector.tensor_scalar_add(res, outp, ct['cgb2'])
                nc.sync.dma_start(out=outd[:, c0:c0 + bt], in_=res)

    nc.compile()
    return nc


_PROGRAM_CACHE = {}


def kernel(**inputs):
    x = np.asarray(inputs['x'], _f32)
    assert x.shape == (B_TOTAL, 3 * DOF), x.shape
    C = precompute_consts(inputs)
    hostin = build_host_inputs(x)

    key = (BC, BT)
    if key not in _PROGRAM_CACHE:
        _PROGRAM_CACHE[key] = build_program(BC, BT)
    nc = _PROGRAM_CACHE[key]

    from concourse.bass_utils import run_bass_kernel_spmd
    cpack = pack_consts(C)
    in_maps = []
    for i in range(N_CORES):
        m = {'cpack': cpack}
        for n, arr in hostin.items():
            m[n] = np.ascontiguousarray(arr[:, i * BC:(i + 1) * BC])
        in_maps.append(m)
    res = run_bass_kernel_spmd(nc, in_maps, core_ids=list(range(N_CORES))).results
    out = np.concatenate([res[i]['out'] for i in range(N_CORES)], axis=1)  # [7,B]
    return np.ascontiguousarray(out.T).astype(_f32)
